# revision 1
# baseline (speedup 1.0000x reference)
"""Self-contained Trainium2 Bass kernel for one GPT-2-style transformer
block (B=4, T=2048, C=768, 12 heads, exact-erf GELU MLP), running SPMD on
8 NeuronCores.

Entry point: kernel(**inputs) -> np.ndarray  (full [4, 2048, 768] output).
"""

import sys

sys.path.insert(0, "/opt/trn_rl_repo")




import concourse.tile as tile
from concourse.vector_clock import ScopedClock, VectorClock


def _patched_drain_and_barrier(self, tick_clock, wait_clock):
    nc = self.nc
    gc = tick_clock.global_clock

    # One NOP per active processor, each carrying at most one sem wait.
    for proc in range(len(gc)):
        tick = gc[proc]
        if tick <= 0:
            continue
        vc = VectorClock()
        vc.require_at_least(proc, tick)
        nop = nc.sync.nop(nofuse=True)
        wait_clock.add_sem_waits(nop.ins, ScopedClock({None: vc}))

    nc.sync.drain()

    nc.all_engine_barrier()
    assert self.sems is not None
    popped = nc._tile_sem_poison_stack.pop()
    assert popped is self._sem_poison
    nc.clear_and_free_semaphores(list(self.sems.allocated().values()))
    nc.all_engine_barrier()


import json

import concourse.bass as bass_mod

_WSPLIT = [0]


def _split_waits_json(bir: bytes) -> bytes:
    """walrus here accepts at most ONE sync wait per instruction; hoist the
    extras onto same-engine NoOps inserted right before the instruction."""
    j = json.loads(bir)
    changed = False
    for f in j.get("functions", []):
        for b in f.get("blocks", []):
            out = []
            for inst in b.get("instructions", []):
                si = inst.get("sync_info")
                waits = (si or {}).get("on_wait") or []
                if len(waits) > 1:
                    changed = True
                    for w in waits[:-1]:
                        _WSPLIT[0] += 1
                        out.append({
                            "debug": inst.get("debug", 0),
                            "engine": inst["engine"],
                            "ins": [],
                            "outs": [],
                            "name": f"I-wsplit-{_WSPLIT[0]}",
                            "opcode": "NoOp",
                            "sync_info": {"on_update": [], "on_wait": [w]},
                        })
                    si["on_wait"] = [waits[-1]]
                out.append(inst)
            b["instructions"] = out
    if not changed:
        return bir
    return json.dumps(j).encode()


_orig_to_json_bytes = bass_mod.Bass.to_json_bytes


def _patched_to_json_bytes(self):
    return _split_waits_json(_orig_to_json_bytes(self))


def apply():
    tile.TileContext._drain_and_barrier = _patched_drain_and_barrier
    bass_mod.Bass.to_json_bytes = _patched_to_json_bytes


apply()




import numpy as np
import concourse.bass as bass
import concourse.tile as tile
from concourse import mybir

F32 = mybir.dt.float32
F32R = mybir.dt.float32r
F16 = mybir.dt.float16
AF = mybir.ActivationFunctionType
OP = mybir.AluOpType

C = 768
CC = 6
D = 64
H = 6
HID = 3072
HC = 24
EPS = 1e-5
SCALE = 0.125   # 1/sqrt(64)
EXPB = -4.0     # uniform exp bias; cancels in softmax normalization


def build(n_cores: int, T: int, phase_limit: int = 99):
    QT = T // 512
    TC = T // 128
    TH = T // 2
    QT2 = TH // 512

    nc = bass.Bass("TRN2", target_bir_lowering=False, debug=False,
                   num_devices=n_cores)

    dp = lambda name, shape, dt=F32, out=False: nc.declare_dram_parameter(
        name, shape, dt, isOutput=out)

    xT = dp("xT", [C, T])
    xhalfT = dp("xhalfT", [C, TH])
    wqk = dp("wqk", [C, 768], F16)     # [:, :384]=Q cols, [:, 384:]=K cols
    wv = dp("wv", [C, 384], F16)
    wo = dp("wo", [384, C], F16)
    ln1g = dp("ln1g", [C]); ln1b = dp("ln1b", [C])
    ln2g = dp("ln2g", [C]); ln2b = dp("ln2b", [C])
    wfc = dp("wfc", [C, HID], F16)
    wproj = dp("wproj", [HID, C], F16)
    outT = dp("outT", [C, TH], out=True)

    attn_bounce = nc.dram_tensor("attn_bounce", [2, C, TH], F32)
    rs_out = nc.dram_tensor("rs_out", [1, C, TH], F32)
    groups = [[2 * i, 2 * i + 1] for i in range(n_cores // 2)]

    with (
        nc.allow_low_precision(reason="fp16/f32r matmuls"),
        tile.TileContext(nc) as tc,
        tc.tile_pool(name="const", bufs=1) as constp,
        tc.tile_pool(name="stats", bufs=1) as stats,
        tc.tile_pool(name="ynorm", bufs=2) as ynorm,
        tc.tile_pool(name="xsqp", bufs=1) as xsqp,
        tc.tile_pool(name="tmp2", bufs=2) as tmp2,
        tc.tile_pool(name="obuf", bufs=3) as obuf,
    ):
        # ---------------- constants ----------------
        mask_sb = constp.tile([128, 896], F16)
        nc.gpsimd.memset(mask_sb, 1.0)
        # mask[i, jj] = 1.0 iff i <= jj - 384  (predicate -i + jj - 384 >= 0)
        nc.gpsimd.affine_select(
            out=mask_sb, in_=mask_sb, compare_op=OP.is_ge, fill=0.0,
            base=-384, pattern=[[1, 896]], channel_multiplier=-1)
        ones_col = constp.tile([128, 1], F32R)
        nc.vector.memset(ones_col.bitcast(F32), 1.0)
        ones_row = constp.tile([1, 128], F32R)
        nc.vector.memset(ones_row.bitcast(F32), 1.0)
        ones_p64 = constp.tile([65, 64], F32R)
        nc.vector.memset(ones_p64.bitcast(F32), 1.0)
        eps_t = constp.tile([1, 1], F32)
        nc.vector.memset(eps_t, EPS)
        expb_t = constp.tile([128, 1], F32)
        nc.vector.memset(expb_t, EXPB)
        g1c = constp.tile([128, CC], F32); b1c = constp.tile([128, CC], F32)
        g2c = constp.tile([128, CC], F32); b2c = constp.tile([128, CC], F32)
        for k in range(CC):
            nc.sync.dma_start(out=g1c[:, k:k + 1], in_=ln1g[128 * k:128 * (k + 1)])
            nc.sync.dma_start(out=b1c[:, k:k + 1], in_=ln1b[128 * k:128 * (k + 1)])
            nc.sync.dma_start(out=g2c[:, k:k + 1], in_=ln2g[128 * k:128 * (k + 1)])
            nc.sync.dma_start(out=b2c[:, k:k + 1], in_=ln2b[128 * k:128 * (k + 1)])

        def layernorm_qt(x_f32, x_f32r, out_sb, oslice, g_col, b_col, st, bc):
            """One 512-token LN tile.  x_f32: [128, CC, 512] fp32-view for DVE,
            x_f32r: same data as F32R views (list of 6 APs) for PE stats.
            Writes out_sb[:, k, oslice] (F16)."""
            xsq = xsqp.tile([128, CC, 512], F32R, tag="xsq")
            for k in range(CC):
                nc.vector.tensor_mul(xsq[:, k, :], x_f32[k], x_f32[k])
            for k in range(CC):
                nc.tensor.matmul(st[:, 0, :], ones_col, x_f32r[k],
                                 start=(k == 0), stop=(k == CC - 1))
            for k in range(CC):
                nc.tensor.matmul(st[:, 1, :], ones_col, xsq[:, k, :],
                                 start=(k == 0), stop=(k == CC - 1))
            m_sb = stats.tile([1, 512], F32R, tag="m")
            s2_sb = stats.tile([1, 512], F32, tag="s2")
            nc.vector.tensor_scalar_mul(m_sb, st[:, 0, :], 1.0 / C)
            nc.vector.tensor_scalar_mul(s2_sb, st[:, 1, :], 1.0 / C)
            var_sb = stats.tile([1, 512], F32, tag="var")
            msq = stats.tile([1, 512], F32, tag="msq")
            nc.vector.tensor_mul(msq, m_sb.bitcast(F32), m_sb.bitcast(F32))
            nc.vector.tensor_sub(var_sb, s2_sb, msq)
            sd = stats.tile([1, 512], F32, tag="sd")
            nc.scalar.activation(sd, var_sb, AF.Sqrt, bias=eps_t)
            rstd = stats.tile([1, 512], F32R, tag="rstd")
            nc.vector.reciprocal(rstd, sd)
            nc.tensor.matmul(bc[:, 0, :], ones_row, m_sb, start=True, stop=True)
            nc.tensor.matmul(bc[:, 1, :], ones_row, rstd, start=True, stop=True)
            for k in range(CC):
                t1 = tmp2.tile([128, 512], F32, tag="t1")
                nc.vector.tensor_sub(t1, x_f32[k], bc[:, 0, :])
                t2 = tmp2.tile([128, 512], F32, tag="t2")
                nc.vector.tensor_mul(t2, t1, bc[:, 1, :])
                nc.vector.tensor_scalar(
                    out=out_sb[:, k, oslice], in0=t2,
                    scalar1=g_col[:, k:k + 1], scalar2=b_col[:, k:k + 1],
                    op0=OP.mult, op1=OP.add)

        # ================= attention half =================
        with tc.tile_pool(name="attA", bufs=1) as attA:
            wqk_sb = attA.tile([128, CC, 768], F16)
            wv_sb = attA.tile([128, CC, 384], F16)
            wo_sb = attA.tile([128, 3, 768], F16)
            for k in range(CC):
                nc.sync.dma_start(out=wqk_sb[:, k, :], in_=wqk[128 * k:128 * (k + 1), :])
                nc.sync.dma_start(out=wv_sb[:, k, :], in_=wv[128 * k:128 * (k + 1), :])
            for k in range(3):
                nc.sync.dma_start(out=wo_sb[:, k, :], in_=wo[128 * k:128 * (k + 1), :])

            ln1xT = attA.tile([128, CC, T], F16)
            with (
                tc.tile_pool(name="xstream", bufs=2) as xstream,
                tc.tile_pool(name="ps_ln1", bufs=1, space="PSUM") as ps_ln1,
            ):
                for qt in range(QT):
                    q0 = qt * 512
                    xt = xstream.tile([128, CC, 512], F32R, tag="xt")
                    for k in range(CC):
                        nc.scalar.dma_start(
                            out=xt[:, k, :],
                            in_=xT[128 * k:128 * (k + 1), q0:q0 + 512].bitcast(F32R))
                    st = ps_ln1.tile([1, 2, 512], F32, tag="st")
                    bc = ps_ln1.tile([128, 2, 512], F32, tag="bc")
                    layernorm_qt(
                        [xt.bitcast(F32)[:, k, :] for k in range(CC)],
                        [xt[:, k, :] for k in range(CC)],
                        ln1xT, slice(q0, q0 + 512), g1c, b1c, st, bc)

            def dump16(chunks):
                for k in range(CC):
                    ob = obuf.tile([128, TH], F32, tag="dump")
                    nc.vector.tensor_copy(ob, chunks[k])
                    nc.sync.dma_start(out=outT[128 * k:128 * (k + 1), :], in_=ob)

            if phase_limit <= 1:
                dump16([ln1xT[:, k, 0:TH] for k in range(CC)])
                return nc

            # qkv projections
            qkT = attA.tile([128, CC, T], F16)
            v_aug = attA.tile([128, TC, H, 65], F16)
            nc.gpsimd.memset(v_aug, 1.0)
            with (
                tc.tile_pool(name="ps_pqk", bufs=3, space="PSUM") as ps_pqk,
                tc.tile_pool(name="ps_pv", bufs=2, space="PSUM") as ps_pv,
            ):
                for qt in range(QT):
                    q0 = qt * 512
                    for co in range(CC):
                        pq = ps_pqk.tile([128, 512], F32, tag="pqk")
                        for ci in range(CC):
                            nc.tensor.matmul(pq, wqk_sb[:, ci, 128 * co:128 * (co + 1)],
                                             ln1xT[:, ci, q0:q0 + 512],
                                             start=(ci == 0), stop=(ci == CC - 1))
                        nc.any.tensor_copy(qkT[:, co, q0:q0 + 512], pq)
                for t in range(TC):
                    pv = ps_pv.tile([128, 384], F32, tag="pv")
                    for ci in range(CC):
                        nc.tensor.matmul(pv, ln1xT[:, ci, 128 * t:128 * (t + 1)],
                                         wv_sb[:, ci, :],
                                         start=(ci == 0), stop=(ci == CC - 1))
                    nc.any.tensor_copy(v_aug[:, t, :, 0:64],
                                       pv.rearrange("p (h d) -> p h d", h=H))

            if phase_limit <= 2:
                dump16([qkT[:, k, 0:TH] for k in range(CC)])
                return nc

            # attention
            yT = attA.tile([128, 3, T], F16)
            with (
                tc.tile_pool(name="expp", bufs=3) as expp,
                tc.tile_pool(name="ps_s", bufs=2, space="PSUM") as ps_s,
                tc.tile_pool(name="ps_y", bufs=2, space="PSUM") as ps_y,
                tc.tile_pool(name="ps_bc", bufs=1, space="PSUM") as ps_bc,
            ):
                for qt in range(QT):
                    q0 = qt * 512
                    NKC = 4 * (qt + 1)
                    for h in range(H):
                        hp = 64 * (h % 2)
                        qc = h // 2
                        kch = 3 + h // 2
                        py = ps_y.tile([65, 512], F32, tag="py")
                        for kg in range(NKC // 2):
                            psm = ps_s.tile([128, 2, 512], F32, tag="ps")
                            for j in range(2):
                                kc = 2 * kg + j
                                nc.tensor.matmul(
                                    psm[:, j, :],
                                    qkT[hp:hp + 64, kch, 128 * kc:128 * (kc + 1)],
                                    qkT[hp:hp + 64, qc, q0:q0 + 512],
                                    start=True, stop=True)
                            ex = expp.tile([128, 2, 512], F16, tag="ex")
                            nc.scalar.activation(
                                ex.rearrange("p a b -> p (a b)"),
                                psm.rearrange("p a b -> p (a b)"),
                                AF.Exp, scale=SCALE, bias=expb_t)
                            for j in range(2):
                                dd = 2 * kg + j - 4 * qt
                                if dd >= 0:
                                    delta = 128 * dd
                                    nc.vector.tensor_mul(
                                        ex[:, j, :], ex[:, j, :],
                                        mask_sb[:, 384 - delta:896 - delta])
                            for j in range(2):
                                kc = 2 * kg + j
                                nc.tensor.matmul(py, v_aug[:, kc, h, :], ex[:, j, :],
                                                 start=(kc == 0), stop=(kc == NKC - 1))
                        yun = ynorm.tile([65, 512], F32, tag="yun")
                        nc.vector.tensor_copy(yun, py)
                        rec = ynorm.tile([65, 512], F32R, tag="rec")
                        nc.vector.reciprocal(rec[64:65, :], yun[64:65, :])
                        pbc = ps_bc.tile([64, 512], F32, tag="pbc")
                        nc.tensor.matmul(pbc, ones_p64[64:65, :], rec[64:65, :],
                                         start=True, stop=True)
                        yn = ynorm.tile([64, 512], F16, tag="yn")
                        nc.vector.tensor_mul(yn, yun[0:64, :], pbc)
                        nc.gpsimd.dma_start(
                            out=yT[hp:hp + 64, h // 2, q0:q0 + 512], in_=yn)

            if phase_limit <= 3:
                dump16([yT[:, k % 3, 0:TH] for k in range(CC)])
                return nc

            # output projection -> DRAM bounce -> pair ReduceScatter
            with tc.tile_pool(name="ps_po", bufs=3, space="PSUM") as ps_po:
                for qt in range(QT):
                    q0 = qt * 512
                    half = qt // (QT // 2)
                    qoff = (qt % (QT // 2)) * 512
                    for co in range(CC):
                        po = ps_po.tile([128, 512], F32, tag="po")
                        for ci in range(3):
                            nc.tensor.matmul(po, wo_sb[:, ci, 128 * co:128 * (co + 1)],
                                             yT[:, ci, q0:q0 + 512],
                                             start=(ci == 0), stop=(ci == 2))
                        o_sb = obuf.tile([128, 512], F32, tag="o")
                        nc.any.tensor_copy(o_sb, po)
                        nc.sync.dma_start(
                            out=attn_bounce.ap()[half, 128 * co:128 * (co + 1),
                                                 qoff:qoff + 512],
                            in_=o_sb)
            nc.gpsimd.collective_compute(
                "ReduceScatter", OP.add, replica_groups=groups,
                ins=[attn_bounce.ap().opt()], outs=[rs_out.ap().opt()])

        if phase_limit <= 4:
            for k in range(CC):
                ob = obuf.tile([128, TH], F32, tag="dump")
                rsx = tmp2.tile([128, TH], F32, tag="rsx")
                nc.sync.dma_start(out=rsx, in_=rs_out.ap()[0, 128 * k:128 * (k + 1), :])
                nc.vector.tensor_copy(ob, rsx)
                nc.sync.dma_start(out=outT[128 * k:128 * (k + 1), :], in_=ob)
            return nc

        # ================= MLP half =================
        with tc.tile_pool(name="mlpA", bufs=1) as mlpA:
            x1 = mlpA.tile([128, CC, TH], F32)
            res_p_cm = tc.tile_pool(name="res_p", bufs=1)
            res_tmp_cm = tc.tile_pool(name="res_tmp", bufs=2)
            res_p = res_p_cm.__enter__(); res_tmp = res_tmp_cm.__enter__()
            x1r = res_p.tile([128, CC, TH], F32R)
            for k in range(CC):
                xh_sb = res_tmp.tile([128, TH], F32, tag="xh")
                nc.sync.dma_start(out=xh_sb, in_=xhalfT[128 * k:128 * (k + 1), :])
                rs_sb = res_tmp.tile([128, TH], F32, tag="rs")
                nc.sync.dma_start(out=rs_sb,
                                  in_=rs_out.ap()[0, 128 * k:128 * (k + 1), :])
                nc.vector.tensor_add(x1[:, k, :], xh_sb, rs_sb)
                nc.vector.tensor_copy(x1r[:, k, :], x1[:, k, :])

            def dump16b(chunks):
                for k in range(CC):
                    ob = obuf.tile([128, TH], F32, tag="dump")
                    nc.vector.tensor_copy(ob, chunks[k])
                    nc.sync.dma_start(out=outT[128 * k:128 * (k + 1), :], in_=ob)

            ln2xT = mlpA.tile([128, CC, TH], F16)
            with tc.tile_pool(name="ps_ln2", bufs=1, space="PSUM") as ps_ln2:
                for qt in range(QT2):
                    q0 = qt * 512
                    st = ps_ln2.tile([1, 2, 512], F32, tag="st")
                    bc = ps_ln2.tile([128, 2, 512], F32, tag="bc")
                    layernorm_qt(
                        [x1[:, k, q0:q0 + 512] for k in range(CC)],
                        [x1r[:, k, q0:q0 + 512] for k in range(CC)],
                        ln2xT, slice(q0, q0 + 512), g2c, b2c, st, bc)

            if phase_limit <= 5:
                dump16b([ln2xT[:, k, :] for k in range(CC)])
                return nc

            res_tmp_cm.__exit__(None, None, None)
            res_p_cm.__exit__(None, None, None)

            hgelu = mlpA.tile([128, HC, TH], F16)
            with (
                tc.tile_pool(name="wstream", bufs=3) as wstream,
                tc.tile_pool(name="ps_ph", bufs=2, space="PSUM") as ps_ph,
            ):
                for hc in range(HC):
                    wfc_t = wstream.tile([128, CC, 128], F16, tag="wfc")
                    for ci in range(CC):
                        nc.sync.dma_start(
                            out=wfc_t[:, ci, :],
                            in_=wfc[128 * ci:128 * (ci + 1), 128 * hc:128 * (hc + 1)])
                    for q2 in range(QT2):
                        q0 = q2 * 512
                        ph = ps_ph.tile([128, 512], F32, tag="ph")
                        for ci in range(CC):
                            nc.tensor.matmul(ph, wfc_t[:, ci, :],
                                             ln2xT[:, ci, q0:q0 + 512],
                                             start=(ci == 0), stop=(ci == CC - 1))
                        nc.scalar.activation(hgelu[:, hc, q0:q0 + 512], ph, AF.Gelu)

            if phase_limit <= 6:
                dump16b([hgelu[:, k, :] for k in range(CC)])
                return nc

            with (
                tc.tile_pool(name="wp_all_p", bufs=1) as wp_all_p,
                tc.tile_pool(name="ps_pm", bufs=2, space="PSUM") as ps_pm,
            ):
                wp_all = wp_all_p.tile([128, HC, 768], F16)
                for hc in range(HC):
                    nc.sync.dma_start(out=wp_all[:, hc, :],
                                      in_=wproj[128 * hc:128 * (hc + 1), :])
                for q2 in range(QT2):
                    q0 = q2 * 512
                    for co in range(CC):
                        pm = ps_pm.tile([128, 512], F32, tag="pm")
                        for hc in range(HC):
                            nc.tensor.matmul(pm,
                                             wp_all[:, hc, 128 * co:128 * (co + 1)],
                                             hgelu[:, hc, q0:q0 + 512],
                                             start=(hc == 0), stop=(hc == HC - 1))
                        ob = obuf.tile([128, 512], F32, tag="ob")
                        nc.vector.tensor_add(ob, pm, x1[:, co, q0:q0 + 512])
                        nc.sync.dma_start(
                            out=outT[128 * co:128 * (co + 1), q0:q0 + 512], in_=ob)

    return nc


# ---------------- host-side sharding ----------------

def shard_inputs(inputs, n_cores=8):
    x = np.asarray(inputs["x"])
    W_attn = np.asarray(inputs["W_attn"])
    W_o = np.asarray(inputs["W_o"])
    B, T, _ = x.shape
    TH = T // 2
    f16 = lambda a: np.ascontiguousarray(a, dtype=np.float16)
    in_maps = []
    for c in range(n_cores):
        b, p = c // 2, c % 2
        Wq = W_attn[:, 384 * p:384 * (p + 1)]
        Wk = W_attn[:, 768 + 384 * p:768 + 384 * (p + 1)]
        Wv = W_attn[:, 1536 + 384 * p:1536 + 384 * (p + 1)]
        in_maps.append({
            "xT": np.ascontiguousarray(x[b].T),
            "xhalfT": np.ascontiguousarray(x[b, TH * p:TH * (p + 1)].T),
            "wqk": f16(np.concatenate([Wq, Wk], axis=1)),
            "wv": f16(Wv),
            "wo": f16(W_o[384 * p:384 * (p + 1), :]),
            "ln1g": np.asarray(inputs["ln1_g"]),
            "ln1b": np.asarray(inputs["ln1_b"]),
            "ln2g": np.asarray(inputs["ln2_g"]),
            "ln2b": np.asarray(inputs["ln2_b"]),
            "wfc": f16(inputs["W_fc"]),
            "wproj": f16(inputs["W_proj"]),
        })
    return in_maps


def unshard(results, n_cores=8, T=2048):
    TH = T // 2
    out = np.empty((n_cores // 2, T, C), np.float32)
    for c in range(n_cores):
        b, p = c // 2, c % 2
        out[b, TH * p:TH * (p + 1)] = results[c]["outT"].T
    return out


_CACHED = {}


def kernel(**inputs):
    import numpy as np
    from concourse.bass_utils import run_bass_kernel_spmd

    n_cores, T = 8, 2048
    if "nc" not in _CACHED:
        _CACHED["nc"] = build(n_cores=n_cores, T=T)
    nc = _CACHED["nc"]
    in_maps = shard_inputs(inputs, n_cores=n_cores)
    res = run_bass_kernel_spmd(nc, in_maps, core_ids=list(range(n_cores)))
    return unshard(res.results, n_cores=n_cores, T=T)



# revision 14
# speedup vs baseline: 330.1587x; 330.1587x over previous
"""Self-contained Trainium2 Bass kernel for one GPT-2-style transformer
block (B=4, T=2048, C=768, 12 heads, exact-erf GELU MLP), running SPMD on
8 NeuronCores.

Entry point: kernel(**inputs) -> np.ndarray  (full [4, 2048, 768] output).
"""

import sys

sys.path.insert(0, "/opt/trn_rl_repo")




import concourse.tile as tile
from concourse.vector_clock import ScopedClock, VectorClock


def _patched_drain_and_barrier(self, tick_clock, wait_clock):
    nc = self.nc
    gc = tick_clock.global_clock

    # One NOP per active processor, each carrying at most one sem wait.
    for proc in range(len(gc)):
        tick = gc[proc]
        if tick <= 0:
            continue
        vc = VectorClock()
        vc.require_at_least(proc, tick)
        nop = nc.sync.nop(nofuse=True)
        wait_clock.add_sem_waits(nop.ins, ScopedClock({None: vc}))

    nc.sync.drain()

    nc.all_engine_barrier()
    assert self.sems is not None
    popped = nc._tile_sem_poison_stack.pop()
    assert popped is self._sem_poison
    nc.clear_and_free_semaphores(list(self.sems.allocated().values()))
    nc.all_engine_barrier()


import json

import concourse.bass as bass_mod

_WSPLIT = [0]


def _split_waits_json(bir: bytes) -> bytes:
    """walrus here accepts at most ONE sync wait per instruction; hoist the
    extras onto same-engine NoOps inserted right before the instruction."""
    j = json.loads(bir)
    changed = False
    for f in j.get("functions", []):
        for b in f.get("blocks", []):
            out = []
            for inst in b.get("instructions", []):
                si = inst.get("sync_info")
                waits = (si or {}).get("on_wait") or []
                if len(waits) > 1:
                    changed = True
                    for w in waits[:-1]:
                        _WSPLIT[0] += 1
                        out.append({
                            "debug": inst.get("debug", 0),
                            "engine": inst["engine"],
                            "ins": [],
                            "outs": [],
                            "name": f"I-wsplit-{_WSPLIT[0]}",
                            "opcode": "NoOp",
                            "sync_info": {"on_update": [], "on_wait": [w]},
                        })
                    si["on_wait"] = [waits[-1]]
                out.append(inst)
            b["instructions"] = out
    if not changed:
        return bir
    return json.dumps(j).encode()


_orig_to_json_bytes = bass_mod.Bass.to_json_bytes


def _patched_to_json_bytes(self):
    return _split_waits_json(_orig_to_json_bytes(self))


def apply():
    tile.TileContext._drain_and_barrier = _patched_drain_and_barrier
    bass_mod.Bass.to_json_bytes = _patched_to_json_bytes


apply()




import numpy as np
import concourse.bass as bass
import concourse.tile as tile
from concourse import mybir

F32 = mybir.dt.float32
F32R = mybir.dt.float32r
F16 = mybir.dt.float16
AF = mybir.ActivationFunctionType
OP = mybir.AluOpType

C = 768
CC = 6
D = 64
H = 6
HID = 3072
HC = 24
EPS = 1e-5
SCALE = 0.125   # 1/sqrt(64)
EXPB = -4.0     # uniform exp bias; cancels in softmax normalization


def build(n_cores: int, T: int, phase_limit: int = 99, reps: int = 1):
    QT = T // 512
    TC = T // 128
    TH = T // 2
    QT2 = TH // 512

    nc = bass.Bass("TRN2", target_bir_lowering=False, debug=False,
                   num_devices=n_cores)

    dp = lambda name, shape, dt=F32, out=False: nc.declare_dram_parameter(
        name, shape, dt, isOutput=out)

    xT = dp("xT", [C, T])
    xhalfT = dp("xhalfT", [C, TH])
    wqk = dp("wqk", [C, 768], F16)     # [:, :384]=Q cols, [:, 384:]=K cols
    wv = dp("wv", [C, 384], F16)
    wo = dp("wo", [384, C], F16)
    ln1g = dp("ln1g", [C]); ln1b = dp("ln1b", [C])
    ln2g = dp("ln2g", [C]); ln2b = dp("ln2b", [C])
    wfc = dp("wfc", [C, HID], F16)
    wproj = dp("wproj", [HID, C], F16)
    outT = dp("outT", [C, TH], out=True)

    # column-split bounce buffers: A holds token-cols [0,512) of each half
    # (written by qt0/qt2), B holds cols [512,1024) (qt1/qt3).  Splitting lets
    # the pair ReduceScatter for A start right after qt2 and overlap qt3.
    bounceA = nc.dram_tensor("bounceA", [2, C, 512], F32)
    bounceB = nc.dram_tensor("bounceB", [2, C, 512], F32)
    rsA = nc.dram_tensor("rsA", [1, C, 512], F32)
    rsB = nc.dram_tensor("rsB", [1, C, 512], F32)
    groups = [[2 * i, 2 * i + 1] for i in range(n_cores // 2)]

    with (
        nc.allow_low_precision(reason="fp16/f32r matmuls"),
        tile.TileContext(nc) as tc,
    ):
     for _rep in range(reps):
      with (
        tc.tile_pool(name="const", bufs=1) as constp,
        tc.tile_pool(name="stats", bufs=1) as stats,
        tc.tile_pool(name="xsqp", bufs=1) as xsqp,
        tc.tile_pool(name="tmp2", bufs=2) as tmp2,
        tc.tile_pool(name="obuf", bufs=2) as obuf,
        # PSUM budget (8 banks): ps_s tag "ps" 2x[128,2,512] = 4 banks,
        # ps_b tag "b" 4x[128,512] = 4 banks.
        tc.tile_pool(name="ps_s", bufs=2, space="PSUM") as ps_s,
        tc.tile_pool(name="ps_b", bufs=4, space="PSUM") as ps_b,
    ):
        # ---------------- constants ----------------
        mask_sb = constp.tile([128, 896], F16)
        nc.gpsimd.memset(mask_sb, 1.0)
        # mask[i, jj] = 1.0 iff i <= jj - 384  (predicate -i + jj - 384 >= 0)
        nc.gpsimd.affine_select(
            out=mask_sb, in_=mask_sb, compare_op=OP.is_ge, fill=0.0,
            base=-384, pattern=[[1, 896]], channel_multiplier=-1)
        ones_col = constp.tile([128, 1], F32R)
        nc.vector.memset(ones_col.bitcast(F32), 1.0)
        ones_col16 = constp.tile([128, 1], F16)
        nc.vector.memset(ones_col16, 1.0)
        ones_row = constp.tile([1, 128], F32R)
        nc.vector.memset(ones_row.bitcast(F32), 1.0)
        ones_p64 = constp.tile([65, 64], F32R)
        nc.vector.memset(ones_p64.bitcast(F32), 1.0)
        eps_t = constp.tile([1, 1], F32)
        nc.vector.memset(eps_t, EPS)
        expb_t = constp.tile([128, 1], F32)
        nc.vector.memset(expb_t, EXPB)
        g1c = constp.tile([128, CC], F32); b1c = constp.tile([128, CC], F32)
        g2c = constp.tile([128, CC], F32); b2c = constp.tile([128, CC], F32)
        for k in range(CC):
            nc.sync.dma_start(out=g1c[:, k:k + 1], in_=ln1g[128 * k:128 * (k + 1)])
            nc.sync.dma_start(out=b1c[:, k:k + 1], in_=ln1b[128 * k:128 * (k + 1)])
            nc.sync.dma_start(out=g2c[:, k:k + 1], in_=ln2g[128 * k:128 * (k + 1)])
            nc.sync.dma_start(out=b2c[:, k:k + 1], in_=ln2b[128 * k:128 * (k + 1)])

        def layernorm_qt(x_f32, x_stat, out_sb, oslice, g_col, b_col,
                         ones_stat):
            """One 512-token LN tile.  x_f32: list of 6 [128,512] fp32 views
            for the DVE normalize; x_stat: views for the PE stats matmul
            (F32R or F16, with matching ones_stat).  Writes out_sb (F16)."""
            st_s = ps_b.tile([1, 512], F32, tag="b")
            st_q = ps_b.tile([1, 512], F32, tag="b")
            xsq = xsqp.tile([128, CC, 512], F16, tag="xsq")
            for k in range(CC):
                nc.vector.tensor_mul(xsq[:, k, :], x_f32[k], x_f32[k])
            for k in range(CC):
                nc.tensor.matmul(st_s, ones_stat, x_stat[k],
                                 start=(k == 0), stop=(k == CC - 1))
            for k in range(CC):
                nc.tensor.matmul(st_q, ones_col16, xsq[:, k, :],
                                 start=(k == 0), stop=(k == CC - 1))
            m_sb = stats.tile([1, 512], F32R, tag="m")
            s2_sb = stats.tile([1, 512], F32, tag="s2")
            nc.vector.tensor_scalar_mul(m_sb, st_s, 1.0 / C)
            nc.vector.tensor_scalar_mul(s2_sb, st_q, 1.0 / C)
            var_sb = stats.tile([1, 512], F32, tag="var")
            msq = stats.tile([1, 512], F32, tag="msq")
            nc.vector.tensor_mul(msq, m_sb.bitcast(F32), m_sb.bitcast(F32))
            nc.vector.tensor_sub(var_sb, s2_sb, msq)
            sd = stats.tile([1, 512], F32, tag="sd")
            nc.scalar.activation(sd, var_sb, AF.Sqrt, bias=eps_t)
            rstd = stats.tile([1, 512], F32R, tag="rstd")
            nc.vector.reciprocal(rstd, sd)
            m_bc = ps_b.tile([128, 512], F32, tag="b")
            r_bc = ps_b.tile([128, 512], F32, tag="b")
            nc.tensor.matmul(m_bc, ones_row, m_sb, start=True, stop=True)
            nc.tensor.matmul(r_bc, ones_row, rstd, start=True, stop=True)
            for k in range(CC):
                t1 = tmp2.tile([128, 512], F32, tag="t1")
                nc.vector.tensor_sub(t1, x_f32[k], m_bc)
                t2 = tmp2.tile([128, 512], F32, tag="t2")
                nc.vector.tensor_mul(t2, t1, r_bc)
                nc.vector.tensor_scalar(
                    out=out_sb[:, k, oslice], in0=t2,
                    scalar1=g_col[:, k:k + 1], scalar2=b_col[:, k:k + 1],
                    op0=OP.mult, op1=OP.add)

        # ---------------- weights ----------------
        attA_cm = tc.tile_pool(name="attA", bufs=1)
        attA = attA_cm.__enter__()
        wqk_sb = attA.tile([128, CC, 768], F16)
        wv_sb = attA.tile([128, CC, 384], F16)
        wo_sb = attA.tile([128, 3, 768], F16)
        for k in range(CC):
            nc.sync.dma_start(out=wqk_sb[:, k, :], in_=wqk[128 * k:128 * (k + 1), :])
            nc.sync.dma_start(out=wv_sb[:, k, :], in_=wv[128 * k:128 * (k + 1), :])
        for k in range(3):
            nc.sync.dma_start(out=wo_sb[:, k, :], in_=wo[128 * k:128 * (k + 1), :])
        # residual-half loads: input-independent, issue at t=0 on SP
        xh_sbs = []
        for k in range(CC):
            xh_sb = tmp2.tile([128, TH], F32, tag=f"xh{k}", bufs=1)
            nc.sync.dma_start(out=xh_sb, in_=xhalfT[128 * k:128 * (k + 1), :])
            xh_sbs.append(xh_sb)

        ln1xT = attA.tile([128, CC, T], F16)
        qkT = attA.tile([128, CC, T], F16)
        v_aug = attA.tile([128, TC, H, 65], F16)
        nc.gpsimd.memset(v_aug, 1.0)
        yT = attA.tile([128, 3, T], F16)

        # ========== fused LN1 -> qkv -> attention -> wo pipeline ==========
        for qt in range(QT):
            q0 = qt * 512
            # ---- LN1 chunk ----
            xt = attA.tile([128, CC, 512], F32R, tag="xt", bufs=2)
            for k in range(CC):
                nc.sync.dma_start(
                    out=xt[:, k, :],
                    in_=xT[128 * k:128 * (k + 1), q0:q0 + 512].bitcast(F32R))
            layernorm_qt(
                [xt.bitcast(F32)[:, k, :] for k in range(CC)],
                [xt[:, k, :] for k in range(CC)],
                ln1xT, slice(q0, q0 + 512), g1c, b1c, ones_col)

            # ---- qk projection chunk ----
            for co in range(CC):
                pq = ps_b.tile([128, 512], F32, tag="b")
                for ci in range(CC):
                    nc.tensor.matmul(pq, wqk_sb[:, ci, 128 * co:128 * (co + 1)],
                                     ln1xT[:, ci, q0:q0 + 512],
                                     start=(ci == 0), stop=(ci == CC - 1))
                nc.any.tensor_copy(qkT[:, co, q0:q0 + 512], pq)
            # ---- v projection chunk ----
            for t in range(4 * qt, 4 * (qt + 1)):
                pv = ps_b.tile([128, 384], F32, tag="b")
                for ci in range(CC):
                    nc.tensor.matmul(pv, ln1xT[:, ci, 128 * t:128 * (t + 1)],
                                     wv_sb[:, ci, :],
                                     start=(ci == 0), stop=(ci == CC - 1))
                nc.any.tensor_copy(v_aug[:, t, :, 0:64],
                                   pv.rearrange("p (h d) -> p h d", h=H))

            # ---- attention for query tile qt ----
            NKC = 4 * (qt + 1)
            for h in range(H):
                hp = 64 * (h % 2)
                qc = h // 2
                kch = 3 + h // 2
                py = ps_b.tile([65, 512], F32, tag="b")
                for kg in range(NKC // 2):
                    psm = ps_s.tile([128, 1024], F32, tag="ps")
                    for j in range(2):
                        kc = 2 * kg + j
                        nc.tensor.matmul(
                            psm[:, 512 * j:512 * (j + 1)],
                            qkT[hp:hp + 64, kch, 128 * kc:128 * (kc + 1)],
                            qkT[hp:hp + 64, qc, q0:q0 + 512],
                            start=True, stop=True)
                    ex = attA.tile([128, 1024], F16, tag="ex", bufs=3)
                    nc.scalar.activation(ex, psm, AF.Exp, scale=SCALE,
                                         bias=expb_t)
                    for j in range(2):
                        dd = 2 * kg + j - 4 * qt
                        if dd >= 0:
                            delta = 128 * dd
                            nc.vector.tensor_mul(
                                ex[:, 512 * j:512 * (j + 1)],
                                ex[:, 512 * j:512 * (j + 1)],
                                mask_sb[:, 384 - delta:896 - delta])
                    for j in range(2):
                        kc = 2 * kg + j
                        nc.tensor.matmul(py, v_aug[:, kc, h, :],
                                         ex[:, 512 * j:512 * (j + 1)],
                                         start=(kc == 0), stop=(kc == NKC - 1))
                yun = attA.tile([65, 512], F32, tag="yun", bufs=2)
                nc.vector.tensor_copy(yun, py)
                rec = attA.tile([65, 512], F32R, tag="rec", bufs=2)
                nc.vector.reciprocal(rec[64:65, :], yun[64:65, :])
                pbc = ps_b.tile([64, 512], F32, tag="b")
                nc.tensor.matmul(pbc, ones_p64[64:65, :], rec[64:65, :],
                                 start=True, stop=True)
                yn = attA.tile([64, 512], F16, tag="yn", bufs=2)
                nc.vector.tensor_mul(yn, yun[0:64, :], pbc)
                nc.vector.tensor_copy(yT[hp:hp + 64, h // 2, q0:q0 + 512], yn)

            # ---- wo partial projection for tile qt -> DRAM bounce ----
            half = qt // (QT // 2)
            bounce = bounceA if qt % 2 == 0 else bounceB
            for co in range(CC):
                po = ps_b.tile([128, 512], F32, tag="b")
                for ci in range(3):
                    nc.tensor.matmul(po, wo_sb[:, ci, 128 * co:128 * (co + 1)],
                                     yT[:, ci, q0:q0 + 512],
                                     start=(ci == 0), stop=(ci == 2))
                o_sb = obuf.tile([128, 512], F32, tag="o")
                nc.any.tensor_copy(o_sb, po)
                nc.sync.dma_start(
                    out=bounce.ap()[half, 128 * co:128 * (co + 1), :],
                    in_=o_sb)
            if qt == 2:
                # bounceA complete (qt0 + qt2): reduce it while qt3 runs
                nc.gpsimd.collective_compute(
                    "ReduceScatter", OP.add, replica_groups=groups,
                    ins=[bounceA.ap().opt()], outs=[rsA.ap().opt()])

        attA_cm.__exit__(None, None, None)

        # ================= MLP half =================
        mlpB_cm = tc.tile_pool(name="mlpB", bufs=1)
        mlpB = mlpB_cm.__enter__()
        x1 = mlpB.tile([128, CC, TH], F32)
        ln2xT = mlpB.tile([128, CC, TH], F16)

        def mlp_prologue(q2):
            q0 = q2 * 512
            rs = rsA if q2 == 0 else rsB
            x16 = mlpB.tile([128, CC, 512], F16, tag="x16", bufs=1)
            for k in range(CC):
                rs_sb = tmp2.tile([128, 512], F32, tag="rs")
                nc.sync.dma_start(out=rs_sb,
                                  in_=rs.ap()[0, 128 * k:128 * (k + 1), :])
                nc.vector.tensor_add(x1[:, k, q0:q0 + 512],
                                     xh_sbs[k][:, q0:q0 + 512], rs_sb)
                nc.vector.tensor_copy(x16[:, k, :], x1[:, k, q0:q0 + 512])
            layernorm_qt(
                [x1[:, k, q0:q0 + 512] for k in range(CC)],
                [x16[:, k, :] for k in range(CC)],
                ln2xT, slice(q0, q0 + 512), g2c, b2c, ones_col16)

        # q2=0 residual + LN2 only needs rsA: emit before collective B so it
        # runs in colB's shadow.  wfc on the now-idle SP queue.
        mlp_prologue(0)
        wfc_all = mlpB.tile([128, CC, HID], F16)
        for k in range(CC):
            nc.sync.dma_start(out=wfc_all[:, k, :],
                              in_=wfc[128 * k:128 * (k + 1), :])
        wp_all = mlpB.tile([128, HC, 768], F16)
        for hc in range(HC):
            nc.scalar.dma_start(out=wp_all[:, hc, :],
                                in_=wproj[128 * hc:128 * (hc + 1), :])

        nc.gpsimd.collective_compute(
            "ReduceScatter", OP.add, replica_groups=groups,
            ins=[bounceB.ap().opt()], outs=[rsB.ap().opt()])

        for q2 in range(QT2):
            q0 = q2 * 512
            if q2 > 0:
                mlp_prologue(q2)
            hgelu = mlpB.tile([128, HC, 512], F16, tag="hg")
            for hc in range(HC):
                ph = ps_s.tile([128, 512], F32, tag="ps")
                for ci in range(CC):
                    nc.tensor.matmul(ph, wfc_all[:, ci, 128 * hc:128 * (hc + 1)],
                                     ln2xT[:, ci, q0:q0 + 512],
                                     start=(ci == 0), stop=(ci == CC - 1))
                nc.scalar.activation(hgelu[:, hc, :], ph, AF.Gelu)
            for co in range(CC):
                pm = ps_s.tile([128, 512], F32, tag="ps")
                for hc in range(HC):
                    nc.tensor.matmul(pm,
                                     wp_all[:, hc, 128 * co:128 * (co + 1)],
                                     hgelu[:, hc, :],
                                     start=(hc == 0), stop=(hc == HC - 1))
                ob = obuf.tile([128, 512], F32, tag="ob")
                nc.vector.tensor_add(ob, pm, x1[:, co, q0:q0 + 512])
                nc.sync.dma_start(
                    out=outT[128 * co:128 * (co + 1), q0:q0 + 512], in_=ob)
        mlpB_cm.__exit__(None, None, None)

    return nc


# ---------------- host-side sharding ----------------

def shard_inputs(inputs, n_cores=8):
    x = np.asarray(inputs["x"])
    W_attn = np.asarray(inputs["W_attn"])
    W_o = np.asarray(inputs["W_o"])
    B, T, _ = x.shape
    TH = T // 2
    f16 = lambda a: np.ascontiguousarray(a, dtype=np.float16)
    in_maps = []
    for c in range(n_cores):
        b, p = c // 2, c % 2
        Wq = W_attn[:, 384 * p:384 * (p + 1)]
        Wk = W_attn[:, 768 + 384 * p:768 + 384 * (p + 1)]
        Wv = W_attn[:, 1536 + 384 * p:1536 + 384 * (p + 1)]
        in_maps.append({
            "xT": np.ascontiguousarray(x[b].T),
            "xhalfT": np.ascontiguousarray(x[b, TH * p:TH * (p + 1)].T),
            "wqk": f16(np.concatenate([Wq, Wk], axis=1)),
            "wv": f16(Wv),
            "wo": f16(W_o[384 * p:384 * (p + 1), :]),
            "ln1g": np.asarray(inputs["ln1_g"]),
            "ln1b": np.asarray(inputs["ln1_b"]),
            "ln2g": np.asarray(inputs["ln2_g"]),
            "ln2b": np.asarray(inputs["ln2_b"]),
            "wfc": f16(inputs["W_fc"]),
            "wproj": f16(inputs["W_proj"]),
        })
    return in_maps


def unshard(results, n_cores=8, T=2048):
    TH = T // 2
    out = np.empty((n_cores // 2, T, C), np.float32)
    for c in range(n_cores):
        b, p = c // 2, c % 2
        out[b, TH * p:TH * (p + 1)] = results[c]["outT"].T
    return out


_CACHED = {}


def kernel(**inputs):
    import numpy as np
    from concourse.bass_utils import run_bass_kernel_spmd

    n_cores, T = 8, 2048
    if "nc" not in _CACHED:
        _CACHED["nc"] = build(n_cores=n_cores, T=T)
    nc = _CACHED["nc"]
    in_maps = shard_inputs(inputs, n_cores=n_cores)
    res = run_bass_kernel_spmd(nc, in_maps, core_ids=list(range(n_cores)))
    return unshard(res.results, n_cores=n_cores, T=T)

